# revision 5
# baseline (speedup 1.0000x reference)
"""Trainium2 Bass kernel for nn_ChargesReadoutBlock.

Math: the reference is
    y_l = (x_l @ W_lin_l) / sqrt(256)            (per irrep l = 0e, 1o, 2e)
    p_l = einsum('zui,u,zi->z', y_l, W_tp_l, c_l) / sqrt(2l+1)
    energy = (p_0 + p_1 + p_2) / sqrt(3*256)

Everything is linear, so the two weight stages collapse on the host:
    w_l = W_lin_l @ W_tp_l                       ([256] each)
    energy[z] = sum_k node_feats[z, k] * A[k] * C[z, j(k)]
where A folds w_l and all scalar norms (per-k weight, shared across nodes)
and C[z, j] are the 9 per-node charge components (j = irrep slot of k).

Device kernel (data-parallel over nodes, 8 cores x 8192 nodes):
nodes live on partitions (128) x 64 node-tiles; for each node-tile the
DVE runs 9 fused scalar_tensor_tensor ops
    accum_out[z, j] = sum_u (x_group_j[z, u] * C[z, j]) * A_group_j[u]
(one per irrep component, each a 256-element strided dot) plus one
9-element reduce. All heavy traffic is the single streaming read of
node_feats (contiguous 73.7KB/partition DMA chunks).
"""

import numpy as np

N_NODES = 65536
N_CORES = 8
MUL = 256
K = 9 * MUL            # 2304 features per node
P = 128                # SBUF partitions
N_SHARD = N_NODES // N_CORES   # 8192 nodes per core
T = N_SHARD // P       # 64 node-tiles per partition (node = p*T + t)
G = 8                  # node-tiles per DMA chunk
SQ3 = float(np.sqrt(3.0))
SQ5 = float(np.sqrt(5.0))

_PROGRAM_CACHE = {}
LAST_RESULTS = None    # BassKernelResults of the most recent kernel() call


def build_program(t_tiles=T, g_chunk=G):
    """Build the SPMD Bass program (same program for every core)."""
    import concourse.bass as bass
    import concourse.tile as tile
    from concourse import mybir

    f32 = mybir.dt.float32
    mult = mybir.AluOpType.mult

    nc = bass.Bass(trn_type="TRN2", debug=False, name="charges_readout")
    x = nc.dram_tensor("x", [P, t_tiles * K], f32, kind="ExternalInput").ap()
    arep = nc.dram_tensor("arep", [P, K], f32, kind="ExternalInput").ap()
    ch = nc.dram_tensor("ch", [P, t_tiles * 9], f32, kind="ExternalInput").ap()
    en = nc.dram_tensor("en", [P, t_tiles], f32, kind="ExternalOutput").ap()

    n_chunks = t_tiles // g_chunk

    with tile.TileContext(nc) as tc:
        with tc.tile_pool(name="const", bufs=1) as cpool, \
             tc.tile_pool(name="xp", bufs=2) as xpool, \
             tc.tile_pool(name="scrp", bufs=4) as spool, \
             tc.tile_pool(name="s9p", bufs=4) as s9pool:

            arep_t = cpool.tile([P, K], f32)
            nc.gpsimd.dma_start(out=arep_t[:], in_=arep[:, :])
            ch_t = cpool.tile([P, t_tiles * 9], f32)
            nc.gpsimd.dma_start(out=ch_t[:], in_=ch[:, :])
            en_t = cpool.tile([P, t_tiles], f32)

            # Wait-collectors: absorb the const-DMA completion waits on cheap
            # copy ops so the first scalar_tensor_tensor doesn't accumulate
            # more sync-wait slots than its ISA struct allows.
            dummy = cpool.tile([P, 2], f32)
            nc.vector.tensor_copy(dummy[:, 0:1], arep_t[:, 0:1])
            nc.vector.tensor_copy(dummy[:, 1:2], ch_t[:, 0:1])

            # Strided per-component views of the folded weights: for l=1 the
            # section layout is k = 3u+i, so component i is a stride-3 view.
            a0 = arep_t[:, 0:MUL]
            a1 = arep_t[:, MUL:4 * MUL].rearrange("p (u i) -> p i u", i=3)
            a2 = arep_t[:, 4 * MUL:9 * MUL].rearrange("p (u i) -> p i u", i=5)

            for c in range(n_chunks):
                xg = xpool.tile([P, g_chunk * K], f32)
                nc.gpsimd.dma_start(
                    out=xg[:], in_=x[:, c * g_chunk * K:(c + 1) * g_chunk * K]
                )
                for tsub in range(g_chunk):
                    t_idx = c * g_chunk + tsub
                    base = tsub * K
                    s9 = s9pool.tile([P, 9], f32)
                    scr = spool.tile([P, MUL], f32)
                    x0 = xg[:, base:base + MUL]
                    x1 = xg[:, base + MUL:base + 4 * MUL].rearrange(
                        "p (u i) -> p i u", i=3)
                    x2 = xg[:, base + 4 * MUL:base + 9 * MUL].rearrange(
                        "p (u i) -> p i u", i=5)
                    groups = [(x0, a0, 0)]
                    groups += [(x1[:, i, :], a1[:, i, :], 1 + i) for i in range(3)]
                    groups += [(x2[:, i, :], a2[:, i, :], 4 + i) for i in range(5)]
                    for xi, ai, j in groups:
                        nc.vector.scalar_tensor_tensor(
                            out=scr[:],
                            in0=xi,
                            scalar=ch_t[:, t_idx * 9 + j:t_idx * 9 + j + 1],
                            in1=ai,
                            op0=mult,
                            op1=mult,
                            accum_out=s9[:, j:j + 1],
                        )
                    nc.vector.tensor_reduce(
                        out=en_t[:, t_idx:t_idx + 1],
                        in_=s9[:, 0:9],
                        axis=mybir.AxisListType.X,
                        op=mybir.AluOpType.add,
                    )
            nc.gpsimd.dma_start(out=en[:, :], in_=en_t[:])
    _prune_implied_dma_waits(nc)
    return nc


def _prune_implied_dma_waits(nc):
    """Drop transitively-implied DMA-lane waits from DMACopy instructions.

    Walrus in this toolchain rejects DMAs with more than one sync wait.
    Tile emits (a) the reader-release wait on the DVE proc sem and (b) WAW /
    lane-FIFO waits on DMA completion sems. (b) is redundant whenever an
    earlier DVE instruction already waited on the same (sem >= value) and
    that instruction completed within the DVE wait of (a) — the vector
    clocks make the DMA completion transitively ordered. Tile's wait
    emission is per-proc minimal but not transitively minimal (documented),
    so we do the reduction here, dropping only waits we can prove implied.
    """
    from concourse import mybir

    blocks = nc.m.functions[0].blocks
    # DVE instruction stream in program order with cumulative DVE-sem ticks
    # and the waits each instruction carried.
    dve_sem = None
    dve_stream = []   # (cum_ticks_after, [(sem_name, wait_value), ...])
    cum = 0
    for blk in blocks:
        for inst in blk.instructions:
            if inst.engine != mybir.EngineType.DVE:
                continue
            si = inst.sync_info
            waits = [(w.ant_name, w.wait_value) for w in (si.on_wait or [])] \
                if si else []
            if si and si.on_update:
                for u in si.on_update:
                    if u.ant_name.startswith("DVE"):
                        dve_sem = u.ant_name
                        cum += u.update_value
            dve_stream.append((cum, waits))

    # Per DMA-lane sem: (cumulative completion value, waits the DMA carried),
    # in program order.
    dma_lane = {}
    for blk in blocks:
        for inst in blk.instructions:
            if inst.opcode != "DMACopy":
                continue
            si = inst.sync_info
            if not si or not si.on_update:
                continue
            waits = [(w.ant_name, w.wait_value) for w in (si.on_wait or [])]
            for u in si.on_update:
                lane = dma_lane.setdefault(u.ant_name, [])
                prev = lane[-1][0] if lane else 0
                lane.append((prev + u.update_value, waits))

    def implied_by_dve(sem_name, value, dve_target):
        # DVE sem >= dve_target => the DVE instruction pushing it there (and
        # all earlier DVE instructions, engine is in-order) completed, so
        # every wait they carried is satisfied.
        for cum_after, waits in dve_stream:
            for s, v in waits:
                if s == sem_name and v >= value:
                    return True
            if cum_after >= dve_target:
                break
        return False

    def implied_by(w, other, depth=0):
        """Is wait w (sem >= value) implied by `other` being satisfied?"""
        s, v = w
        os, ov = other
        if os == dve_sem:
            return implied_by_dve(s, v, ov)
        if os in dma_lane:
            # other satisfied => all DMAs on that lane up to value ov
            # completed => their own waits were satisfied beforehand.
            for cum, waits in dma_lane[os]:
                for ww in waits:
                    if ww[0] == s and ww[1] >= v:
                        return True
                    if depth < 2 and implied_by(w, ww, depth + 1):
                        return True
                if cum >= ov:
                    break
        return False

    for blk in blocks:
        for inst in blk.instructions:
            if inst.opcode not in ("DMACopy", "Drain"):
                continue
            si = inst.sync_info
            if not si or not si.on_wait or len(si.on_wait) <= 1:
                continue
            waits = [(w.ant_name, w.wait_value) for w in si.on_wait]
            kept_idx = list(range(len(waits)))
            changed = True
            while changed:
                changed = False
                for i in list(kept_idx):
                    others = [waits[j] for j in kept_idx if j != i]
                    if any(implied_by(waits[i], o) for o in others):
                        kept_idx.remove(i)
                        changed = True
                        break
            si.on_wait = [si.on_wait[i] for i in kept_idx]
    return nc


def _get_program():
    key = (T, G)
    if key not in _PROGRAM_CACHE:
        _PROGRAM_CACHE[key] = build_program()
    return _PROGRAM_CACHE[key]


def fold_weights(W_lin0, W_lin1, W_lin2, W_tp0, W_tp1, W_tp2):
    """Collapse both weight stages + norms into one per-k weight A[2304]."""
    lin_norm = 1.0 / np.sqrt(np.float64(MUL))
    alpha = 1.0 / np.sqrt(3.0 * MUL)
    w0 = W_lin0.astype(np.float64) @ W_tp0.astype(np.float64)
    w1 = W_lin1.astype(np.float64) @ W_tp1.astype(np.float64)
    w2 = W_lin2.astype(np.float64) @ W_tp2.astype(np.float64)
    A = np.empty(K, np.float64)
    A[0:MUL] = w0 * (alpha * lin_norm)
    A[MUL:4 * MUL] = np.repeat(w1 * (alpha * lin_norm / SQ3), 3)
    A[4 * MUL:9 * MUL] = np.repeat(w2 * (alpha * lin_norm / SQ5), 5)
    return A.astype(np.float32)


def kernel(**inputs):
    global LAST_RESULTS
    from concourse.bass_utils import run_bass_kernel_spmd

    node_feats = np.asarray(inputs["node_feats"], dtype=np.float32)
    charges = np.asarray(inputs["charges"], dtype=np.float32)
    A = fold_weights(
        np.asarray(inputs["W_lin0"], dtype=np.float32),
        np.asarray(inputs["W_lin1"], dtype=np.float32),
        np.asarray(inputs["W_lin2"], dtype=np.float32),
        np.asarray(inputs["W_tp0"], dtype=np.float32),
        np.asarray(inputs["W_tp1"], dtype=np.float32),
        np.asarray(inputs["W_tp2"], dtype=np.float32),
    )
    arep = np.ascontiguousarray(np.broadcast_to(A, (P, K)))

    node_feats = np.ascontiguousarray(node_feats)
    charges = np.ascontiguousarray(charges)

    in_maps = []
    for c in range(N_CORES):
        lo, hi = c * N_SHARD, (c + 1) * N_SHARD
        in_maps.append({
            "x": node_feats[lo:hi].reshape(P, T * K),
            "arep": arep,
            "ch": charges[lo:hi].reshape(P, T * 9),
        })

    nc = _get_program()
    res = run_bass_kernel_spmd(nc, in_maps, list(range(N_CORES)))
    LAST_RESULTS = res
    out = np.concatenate(
        [np.asarray(res.results[c]["en"]).reshape(N_SHARD) for c in range(N_CORES)]
    )
    return out
